# revision 91
# baseline (speedup 1.0000x reference)
"""Multi-head attention Trainium2 kernel (8 NeuronCores, SPMD), bf16 edition.

Sharding: 16 (batch, head) pairs -> 2 pairs per core (cores 0-3: batch 0,
cores 4-7: batch 1; each core owns 2 adjacent heads).

Masked keys (mask==1) get score -1e9 in the reference, whose exp underflows
to exactly 0 in f32, so they are dropped on the host before the kernel runs
(~halves attention work). Kept keys are padded to a multiple of 128; a 0/1
"keep" column rides along V and produces the softmax denominator, which
also neutralizes the pads exactly.

The datapath is bf16 (fp8+DoubleRow was implemented and measured at ~7%
output error: quantization noise on the value path does NOT average down
with more keys, because the attention output's magnitude shrinks at the
same sqrt(Nk) rate — so the 0.5 cyc/col fp8 perf mode is unusable here).

Attention is computed TRANSPOSED: attnV^T has lhsT = exp-scores
[128keys, 128q] (full 128-wide stationary) and rhs = V [128keys, 64dims +
keep-col], giving out [128q, 64dims + denominator]. Eight q-blocks
accumulate into two PSUM banks (4 each, via the pending-zero mechanism:
only the very first matmul into a bank carries start=True). Normalization
is folded into the PSUM drain: a DVE reciprocal reads the denominator
column straight from PSUM and the drain copy becomes one broadcast
multiply producing normalized bf16 — same cost as a plain copy. One PE
transpose per 128-q block flips [q, 2*64vd] -> [vd, q] for the
row-parallel output projection. V is projected directly transposed (swap
stationary/moving), so no V transpose pass exists.

exp is the engine bottleneck alongside the PE (~131k PSUM lines/core): it
is split between the Scalar engine (hardware Exp, 10/16 blocks) and the
Vector engine via a custom-DVE op computing exp(x) ~= (1 + x/64)^64
(6 chained squarings; rel err x^2/128, i.e. ~0.1% at typical |x|~0.35,
3.5% at the |x|~2.1 tail — negligible through softmax).

Scheduling: the kb loop streams scores through 3 PSUM score buffers
(projections for K/V/Q drip into tile 0's slots; attnV chunks trail their
exp by 3 slots; the last chunks of each tile spill into the next tile's
first slots so engines drain the exp backlog while the next tile's scores
stream). Output projection chunks ride late slots of following tiles and
one fat fp16 DMA per q-tile writes the per-core partial, summed on host.
"""

import math

import numpy as np
import ml_dtypes

S = 4096
D = 512
NCORES = 8
SCALE = 1.0 / math.sqrt(512.0)
TW = 1024  # q-tile width

TRACE = False
TRACE_KWARGS = {}
LAST_RESULTS = None

_CACHE = {}
_EXP_OP = None


def _get_exp_op():
    """Register (once) a custom DVE op: out = (1 + in0*s0)^64."""
    global _EXP_OP
    if _EXP_OP is not None:
        return _EXP_OP
    from concourse import dve_ops
    from concourse.dve_spec import Spec, Src0, C0, One, sq, lower as dve_lower
    from concourse.dve_uop import DveOpSpec
    from concourse.dve_ops import DveOp, _SUB_OPCODE_FOR_NAME, _CUSTOM_DVE_ROW_BASE

    name = "EXP_SQ6_ANT"
    if name in _SUB_OPCODE_FOR_NAME:
        _EXP_OP = next(op for op in dve_ops.OPS if op.name == name)
        return _EXP_OP
    body = One + Src0 * C0
    for _ in range(6):
        body = sq(body)

    def ref(in0, in1, s0, s1, imm2):
        return (1.0 + in0 * s0) ** 64

    row = _CUSTOM_DVE_ROW_BASE + len(dve_ops.OPS)
    assert row < 0x20, "no free DVE opcode rows"
    _SUB_OPCODE_FOR_NAME[name] = row
    spec = Spec(body=body, reference=ref)
    shas = {}
    for ver in ("v3", "v4"):
        uops = dve_lower(spec, ver=ver)
        shas[ver] = DveOpSpec(name=name, opcode=row, uops=uops,
                              rd1_en=False).sha(ver)
    op = DveOp(name, spec, subdim=False, uops_sha=shas)
    dve_ops.OPS.append(op)
    dve_ops.CUSTOM_DVE_SPECS[name] = spec
    _EXP_OP = op
    return op


# kb indices (mod 16) whose exp runs on the DVE (rest on Scalar/Act).
DVE_KBS = frozenset({1, 4, 7, 9, 11, 14})


def _build(SKP, nzq=False, nzk=False, nzv=False, s=S, tw=TW):
    import concourse.bacc as bacc
    import concourse.mybir as mybir
    import concourse.tile as tile

    exp_op = _get_exp_op()

    KB = SKP // 128
    NQ = s // tw
    NJ = tw // 128  # 128-q blocks per tile
    dt = mybir.dt.float32
    f16 = mybir.dt.float16
    bf = mybir.dt.bfloat16
    Exp = mybir.ActivationFunctionType.Exp
    Ident = mybir.ActivationFunctionType.Identity
    mult = mybir.AluOpType.mult
    add = mybir.AluOpType.add

    nc = bacc.Bacc("TRN2", target_bir_lowering=False, debug=False,
                   num_devices=NCORES)

    ident_d = nc.dram_tensor("identb", [128, 128], bf, kind="ExternalInput").ap()
    xb_d = nc.dram_tensor("xb", [128, 4, s], bf, kind="ExternalInput").ap()
    xkb_d = nc.dram_tensor("xkb", [128, 4, SKP], bf, kind="ExternalInput").ap()
    wq_d = nc.dram_tensor("wqb", [128, 4, 128], bf, kind="ExternalInput").ap()
    wk_d = nc.dram_tensor("wkb", [128, 4, 128], bf, kind="ExternalInput").ap()
    wv_d = nc.dram_tensor("wvb", [128, 4, 128], bf, kind="ExternalInput").ap()
    wo_d = nc.dram_tensor("wob", [128, 512], bf, kind="ExternalInput").ap()
    keep_d = nc.dram_tensor("keepb", [128, KB], bf, kind="ExternalInput").ap()
    bqk_d = nc.dram_tensor("bqk", [128, 2], dt, kind="ExternalInput").ap()
    bv_d = nc.dram_tensor("bvr", [128, 1], dt, kind="ExternalInput").ap()
    out_d = nc.dram_tensor("fpT", [D, s], f16, kind="ExternalOutput").ap()

    with tile.TileContext(nc) as tc:
        with (
            tc.tile_pool(name="const", bufs=1) as const,
            tc.tile_pool(name="big", bufs=1) as big,
            tc.tile_pool(name="exb", bufs=2) as exb,
            tc.tile_pool(name="rawb", bufs=2) as rawb,
            tc.tile_pool(name="recb", bufs=2) as recb,
            tc.tile_pool(name="fob", bufs=3) as fob,
            tc.tile_pool(name="ps_sc", bufs=3, space="PSUM") as ps_sc,
            tc.tile_pool(name="ps_ot", bufs=2, space="PSUM") as ps_ot,
        ):
            ps_pp = ps_sc  # proj/outproj tiles share the scores pool's banks
            # ------------- constants -------------
            wq_t = const.tile([128, 4, 128], bf, name="wq_t")
            wk_t = const.tile([128, 4, 128], bf, name="wk_t")
            wv_t = const.tile([128, 4, 128], bf, name="wv_t")
            wo_t = const.tile([128, 512], bf, name="wo_t")
            id_t = const.tile([128, 128], bf, name="id_t")
            keep_t = const.tile([128, KB], bf, name="keep_t")
            bqk_t = const.tile([128, 2], dt, name="bqk_t")
            bv_t2 = const.tile([128, 1], dt, name="bv_t2")
            nc.sync.dma_start(out=wk_t[:], in_=wk_d)
            if nzq or nzk:
                nc.sync.dma_start(out=bqk_t[:], in_=bqk_d)
            if nzv:
                nc.sync.dma_start(out=bv_t2[:], in_=bv_d)

            xk_t = big.tile([128, 4, SKP], bf, name="xk_t")
            x_t = big.tile([128, 4, s], bf, name="x_t")
            QT8 = big.tile([128, s], bf, name="QT8")
            KT8 = big.tile([128, SKP], bf, name="KT8")
            V8 = big.tile([128, KB, 130], bf, name="V8")
            out2h8 = big.tile([128, s], bf, name="out2h8")

            # input DMAs: keys first (K proj starts earliest), small leading
            # chunks so the first projections launch ASAP
            k0 = min(256, SKP)
            nc.sync.dma_start(out=xk_t[:, :, 0:k0], in_=xkb_d[:, :, 0:k0])
            nc.sync.dma_start(out=wq_t[:], in_=wq_d)
            q0w = min(512, s)
            nc.sync.dma_start(out=x_t[:, :, 0:q0w], in_=xb_d[:, :, 0:q0w])
            if SKP > 256:
                nc.sync.dma_start(out=xk_t[:, :, 256:512],
                                  in_=xkb_d[:, :, 256:512])
            if s > 512:
                nc.sync.dma_start(out=x_t[:, :, 512:1024],
                                  in_=xb_d[:, :, 512:1024])
            nc.sync.dma_start(out=wv_t[:], in_=wv_d)
            nc.sync.dma_start(out=keep_t[:], in_=keep_d)
            for n0 in range(512, SKP, 1024):
                nw = min(1024, SKP - n0)
                nc.sync.dma_start(out=xk_t[:, :, n0:n0 + nw],
                                  in_=xkb_d[:, :, n0:n0 + nw])
            for n0 in range(tw, s, tw):
                nc.sync.dma_start(out=x_t[:, :, n0:n0 + tw],
                                  in_=xb_d[:, :, n0:n0 + tw])
            nc.sync.dma_start(out=wo_t[:], in_=wo_d)
            nc.sync.dma_start(out=id_t[:], in_=ident_d)

            # keep flags into the two per-head denominator columns of V8
            nc.gpsimd.tensor_copy(V8[:, :, 64], keep_t[:])
            nc.gpsimd.tensor_copy(V8[:, :, 129], keep_t[:])

            # ------------- projections (bf16) -------------
            def kproj(n0, w=512):
                w = min(w, SKP - n0)
                pp = ps_pp.tile([128, 512], dt, name="ppk", tag="sc")
                for a in range(4):
                    nc.tensor.matmul(pp[:, 0:w], wk_t[:, a, :],
                                     xk_t[:, a, n0:n0 + w],
                                     start=(a == 0), stop=(a == 3))
                if nzk:
                    nc.scalar.activation(KT8[:, n0:n0 + w], pp[:, 0:w],
                                         Ident, bias=bqk_t[:, 1:2])
                else:
                    nc.scalar.copy(KT8[:, n0:n0 + w], pp[:, 0:w])

            def vproj(kb):
                pp = ps_pp.tile([128, 512], dt, name="ppv", tag="sc")
                for a in range(4):
                    nc.tensor.matmul(pp[:, 0:128],
                                     xk_t[:, a, kb * 128:(kb + 1) * 128],
                                     wv_t[:, a, :],
                                     start=(a == 0), stop=(a == 3))
                dst = V8[:, kb, 0:130].rearrange(
                    "p (g gd) -> p g gd", g=2)[:, :, 0:64]
                src = pp[:, 0:128].rearrange("p (g d) -> p g d", g=2)
                nc.vector.tensor_copy(dst, src)

            def qproj(n0):
                pp = ps_pp.tile([128, 512], dt, name="ppq", tag="sc")
                for a in range(4):
                    nc.tensor.matmul(pp[:, 0:512], wq_t[:, a, :],
                                     x_t[:, a, n0:n0 + 512],
                                     start=(a == 0), stop=(a == 3))
                if nzq:
                    nc.vector.tensor_scalar_add(QT8[:, n0:n0 + 512],
                                                pp[:, 0:512], bqk_t[:, 0:1])
                else:
                    nc.vector.tensor_copy(QT8[:, n0:n0 + 512], pp[:, 0:512])

            # upfront: only what the first scores chunks need; the rest of
            # the projections drip into early tile slots (kproj chunk i
            # covers kb 4i..4i+3, needed from kb-slot 4i; vproj chunk c is
            # needed by attn chunk c at slot c+3).
            kproj(0, 256)
            qproj(0)
            if s > 512:
                qproj(512)
            kdrip = []
            if SKP > 256:
                kdrip.append(lambda: kproj(256, 256))
            kdrip += [(lambda n=n0: kproj(n)) for n0 in range(512, SKP, 512)]
            vdrip = [(lambda k=kb: vproj(k)) for kb in range(KB)]
            qdrip = [(lambda n=n0: qproj(n)) for n0 in range(1024, s, 512)]

            # ------------- streaming attention -------------
            def emit_scores(qq, h, kb, ex_t, dkbs=None):
                hp = h * 64
                sc = ps_sc.tile([128, tw], dt, name="sc", tag="sc")
                for c in range(tw // 512):
                    q0 = qq * tw + c * 512
                    nc.tensor.matmul(sc[:, c * 512:(c + 1) * 512],
                                     KT8[hp:hp + 64, kb * 128:(kb + 1) * 128],
                                     QT8[hp:hp + 64, q0:q0 + 512],
                                     start=True, stop=True)
                dst = ex_t[:, kb, :]
                if (kb in dkbs) if dkbs is not None else \
                        (kb % 16 in DVE_KBS or kb == 16):
                    nc.vector._custom_dve(exp_op, out=dst, in0=sc[:],
                                          s0=SCALE / 64.0)
                else:
                    nc.scalar.activation(dst, sc[:], Exp, scale=SCALE)

            # attnV^T accumulates 8 q-blocks into two PSUM banks (4 blocks
            # per bank via the pending-zero mechanism: only the very first
            # matmul into a bank carries start=True).
            def attn_chunk(kb, h, ex_t, oA, oB):
                hb = h * 65
                for j in range(NJ):
                    o = oA if j < NJ // 2 else oB
                    nc.tensor.matmul(o[:, j % (NJ // 2), :],
                                     ex_t[:, kb, j * 128:(j + 1) * 128],
                                     V8[:, kb, hb:hb + 65],
                                     start=(kb == 0 and j % (NJ // 2) == 0),
                                     stop=(kb == KB - 1),
                                     skip_group_check=True)

            def emit_raws(h, raw, oA, oB):
                # normalization folded into the PSUM->SBUF drain: reciprocal
                # of the denominator column straight from PSUM, then one
                # broadcast-multiply per bank producing normalized bf16.
                # Both heads of a q-tile share `raw` (head h -> cols h*64+).
                rec = recb.tile([128, NJ], dt, name="rec")
                hp = h * 64
                half = NJ // 2
                for hx, oX in ((0, oA), (1, oB)):
                    rsl = rec[:, hx * half:(hx + 1) * half]
                    nc.vector.reciprocal(rsl, oX[:, :, 64])
                    rb = rsl.rearrange("p (j one) -> p j one", one=1) \
                        .broadcast_to([128, half, 64])
                    nc.vector.tensor_tensor(
                        raw[:, hx * half:(hx + 1) * half, hp:hp + 64],
                        oX[:, :, 0:64], rb, op=mult)

            def emit_tpose(qq, raw, j, eng):
                # PE transpose [q, 2*vd] -> [2*vd, q] + engine copy to SBUF
                q0 = qq * tw + j * 128
                tp = ps_sc.tile([128, 128], bf, name="tp", tag="sc")
                nc.tensor.transpose(tp[:], raw[:, j, :], id_t[:])
                dst = out2h8[:, q0:q0 + 128]
                if nzv:
                    nc.scalar.activation(dst, tp[:], Ident,
                                         bias=bv_t2[:, 0:1])
                elif eng == 0:
                    nc.scalar.copy(dst, tp[:])
                else:
                    nc.vector.tensor_copy(dst, tp[:])

            def outproj(qq, i, fo, eng):
                # i = (c-half, cg) chunk index; fo = [128, 4, tw] staging
                # tile. The PSUM drain is split across both engines so the
                # outproj chain is paced at half-copy latency.
                c, cg = i // 4, i % 4
                q0 = qq * tw + c * 512
                po = ps_sc.tile([128, 512], dt, name="po", tag="sc")
                nc.tensor.matmul(po[:], wo_t[:, cg * 128:(cg + 1) * 128],
                                 out2h8[:, q0:q0 + 512],
                                 start=True, stop=True)
                dst = fo[:, cg, c * 512:(c + 1) * 512]
                if eng == 0:
                    nc.scalar.copy(dst, po[:])
                else:
                    nc.vector.tensor_copy(dst, po[:])

            out_r = out_d.rearrange("(cg p) q -> p cg q", p=128)

            def fo_flush(qq, fo):
                nc.sync.dma_start(out=out_r[:, :, qq * tw:(qq + 1) * tw],
                                  in_=fo[:])

            tiles = [(qq, h) for qq in range(NQ) for h in range(2)]
            prev = None
            carry = []
            po_q = []  # pending output-projection chunks: (qq, i, fo)
            raw = None
            for t_idx, (qq, h) in enumerate(tiles):
                ex_t = exb.tile([128, KB, tw], bf, name="ex_t")
                if h == 0:
                    raw = rawb.tile([128, NJ, 128], bf, name="raw")
                oA = ps_ot.tile([128, NJ // 2, 65], dt, name="oA", tag="oT")
                oB = ps_ot.tile([128, NJ // 2, 65], dt, name="oB", tag="oT")
                cur = (qq, h, raw)

                # per-slot extra work inside this tile's kb loop
                slot = {}

                def at(kb, fn):
                    slot.setdefault(kb, []).append(fn)

                # last 3 attn chunks + raw drain of the PREVIOUS tile land in
                # this tile's first slots (the engines finish prev's exps
                # while this tile's scores stream) — no boundary stall
                aoff = 7 if t_idx == 0 else 4
                spill = 6 if KB > 6 else 0
                for c in range(KB - spill):
                    at(c + aoff, (lambda c=c: attn_chunk(c, h, ex_t, oA, oB)))
                if prev is not None:
                    pq, ph, praw = prev
                    for i, fn in enumerate(carry):
                        at(i // 2, fn)
                    if ph == 1:
                        for j in range(NJ):
                            at(3 + j, (lambda j=j: emit_tpose(
                                pq, praw, j, j % 2)))
                        fo = fob.tile([128, 4, tw], f16, name="fo")
                        po_q.extend((pq, i, fo)
                                    for i in range(4 * (tw // 512)))
                carry = [
                    (lambda c=c, hh=h, e=ex_t, a=oA, b=oB:
                     attn_chunk(c, hh, e, a, b))
                    for c in range(KB - spill, KB)
                ] + [(lambda hh=h, r=raw, a=oA, b=oB:
                      emit_raws(hh, r, a, b))]
                # 4 outproj chunks per tile at late slots
                for sl in (12, 13, 14, 15):
                    if po_q:
                        pqq, i, fo = po_q.pop(0)
                        at(sl, (lambda a=pqq, b=i, f=fo:
                                outproj(a, b, f, 1)))
                        if i == 4 * (tw // 512) - 1:
                            at(sl, (lambda a=pqq, f=fo: fo_flush(a, f)))
                if t_idx == 0:
                    # kprojs lead 1/slot, then vprojs 2/slot (keeping ahead
                    # of the attn chunks); qprojs spread over tiles 1-2
                    sl = 0
                    for fn in kdrip:
                        at(sl, fn)
                        sl += 1
                    for i, fn in enumerate(vdrip):
                        at(2, fn)
                elif t_idx in (1, 2) and qdrip:
                    for i in range(3):
                        if qdrip:
                            at(2 + 5 * i, qdrip.pop(0))

                for kb in range(KB):
                    emit_scores(qq, h, kb, ex_t)
                    for fn in slot.pop(kb, []):
                        fn()
                for kb in sorted(slot):
                    for fn in slot.pop(kb, []):
                        fn()
                prev = cur

            # tail: drain the carried attn chunks with the pending outproj
            # chunks interleaved (PE work while the exp backlog drains), then
            # transposes and the final outproj
            pq, ph, praw = prev
            fo = fob.tile([128, 4, tw], f16, name="fo")
            lastq = [(pq, i, fo) for i in range(4 * (tw // 512))]
            for n, fn in enumerate(carry):
                fn()
                if n % 2 == 0 and po_q:
                    pqq, i, fo2 = po_q.pop(0)
                    outproj(pqq, i, fo2, n % 2)
                    if i == 4 * (tw // 512) - 1:
                        fo_flush(pqq, fo2)
            for n, (pqq, i, fo2) in enumerate(po_q):
                outproj(pqq, i, fo2, n % 2)
                if i == 4 * (tw // 512) - 1:
                    fo_flush(pqq, fo2)
            for j in range(NJ):
                emit_tpose(pq, praw, j, j % 2)
            ci = 0
            for c in range(tw // 512):
                for cg in range(4):
                    if ci < len(lastq):
                        outproj(pq, lastq[ci][1], fo, cg % 2)
                        ci += 1
                nc.sync.dma_start(
                    out=out_r[:, :, pq * tw + c * 512:pq * tw + (c + 1) * 512],
                    in_=fo[:, :, c * 512:(c + 1) * 512])

    nc.compile()
    return nc


def _prep_core(x_b, keep_b, Wq, bq, Wk, bk, Wv, bv, Wo, h0, SKP):
    """Host-side input prep for one core (batch slice x_b, head pair h0)."""
    bf = ml_dtypes.bfloat16
    KB = SKP // 128
    sl = slice(h0 * 64, h0 * 64 + 128)

    def wprep(W):
        # [512, 128] -> [p, a, m] with xd = a*128 + p
        return np.ascontiguousarray(
            W[:, sl].astype(bf).reshape(4, 128, 128).transpose(1, 0, 2))

    def xprep(xT, width):
        return np.ascontiguousarray(
            xT.reshape(4, 128, width).transpose(1, 0, 2).astype(bf))

    nk = len(keep_b)
    xk = np.zeros((SKP, D), np.float32)
    xk[:nk] = x_b[keep_b]
    keep = np.zeros((SKP,), np.float32)
    keep[:nk] = 1.0
    return {
        "xb": xprep(x_b.T, x_b.shape[0]),
        "xkb": xprep(xk.T, SKP),
        "wqb": wprep(Wq),
        "wkb": wprep(Wk),
        "wvb": wprep(Wv),
        "wob": np.ascontiguousarray(Wo[sl, :].astype(bf)),
        "keepb": np.ascontiguousarray(keep.reshape(KB, 128).T.astype(bf)),
        "identb": np.eye(128, dtype=np.float32).astype(bf),
        "bqk": np.ascontiguousarray(
            np.stack([bq[sl], bk[sl]], axis=1).astype(np.float32)),
        "bvr": np.ascontiguousarray(bv[sl].astype(np.float32))[:, None],
    }


def kernel(x, mask, Wq, bq, Wk, bk, Wv, bv, Wo, bo):
    global LAST_RESULTS
    from concourse.bass_utils import run_bass_kernel_spmd

    x = np.asarray(x, dtype=np.float32)
    mask = np.asarray(mask)
    Wq, bq = np.asarray(Wq, np.float32), np.asarray(bq, np.float32)
    Wk, bk = np.asarray(Wk, np.float32), np.asarray(bk, np.float32)
    Wv, bv = np.asarray(Wv, np.float32), np.asarray(bv, np.float32)
    Wo, bo = np.asarray(Wo, np.float32), np.asarray(bo, np.float32)
    B = x.shape[0]

    keep_idx = [np.flatnonzero(mask[b] == 0) for b in range(B)]
    SKP = max(256, int(math.ceil(max(len(k) for k in keep_idx) / 128.0)) * 128)

    key = (SKP, bool(bq.any()), bool(bk.any()), bool(bv.any()))
    if key not in _CACHE:
        _CACHE[key] = _build(*key)
    nc = _CACHE[key]

    in_maps = []
    for c in range(NCORES):
        b = c // (NCORES // B)
        h0 = 2 * (c % (NCORES // B))
        in_maps.append(_prep_core(x[b], keep_idx[b], Wq, bq, Wk, bk,
                                  Wv, bv, Wo, h0, SKP))

    res = run_bass_kernel_spmd(nc, in_maps, core_ids=list(range(NCORES)),
                               trace=TRACE, **TRACE_KWARGS)
    LAST_RESULTS = res

    partials = np.stack([np.asarray(r["fpT"], dtype=np.float32)
                         for r in res.results])          # [8, 512, S]
    per_batch = partials.reshape(B, NCORES // B, D, S).sum(axis=1)
    out = per_batch.transpose(0, 2, 1) + bo[None, None, :]
    return np.ascontiguousarray(out.astype(np.float32))
